# revision 55
# baseline (speedup 1.0000x reference)
"""GCN (2-layer GCNConv + global max pool + MLP + log_softmax) on 8 trn2 cores.

Strategy (sharding_hint: partition nodes + incident edges, replicate weights):
  - Nodes are partitioned 6250/core (+pads -> 6272 = 49 tiles of 128),
    degree-sorted per core so per-tile gather width ~= true degree.
  - Aggregation is gather-based: per 128-dst tile, messages are fetched
    with dma_gather (int16 idx, mid-table base).  Gathers are flat
    (slots packed across tile boundaries, j_cap=5 wide) and round-robin
    over the 4 SWDGE queues; Q7 descriptor-gen is the kernel's critical
    resource and overlaps ~2x across queue cpu-pairs.  Each gather ends
    with a sacrificial block of 1 valid + 127 negative indices (ucode
    trims trailing negatives at zero cost; the flaky final descriptor
    lands on the valid sacrificial slot).
  - x is host-prescaled by dinv (xs = dinv*x, bf16, trailing zero row),
    so both layers accumulate with plain strided tensor_reduce adds on
    DVE; pad slots gather exact zeros.  Self-loops are not gathered:
    each tile adds its local (permuted-sequential) xs / h1' rows.
  - h1' (bf16, = dinv*relu(.)) is AllGathered in 4 chunks fired as their
    h-tiles complete; chunk outputs are bump-allocated back-to-back and
    layer 2 gathers straight out of them through a spanning alias tensor
    (no copy).  1-row aliased self-copies register each collective as a
    tracked writer so layer-2 gathers order after all chunks.
  - Pooling: layer-2 h tiles are indirect-scattered into [j-slot, graph]
    layouts (A: tiles < 2T/3, B: rest) and max-reduced over j-slots; A
    reduces under the tail of layer 2.  Per-core pooled rows scatter
    into a [513,256] table, AllReduce(max), then the small MLP +
    log_softmax run replicated on every core.
"""

import numpy as np

import concourse.bass as bass
import concourse.bacc as bacc
import concourse.tile as tile
import concourse.mybir as mybir
from concourse import bass_utils
from concourse.masks import make_identity
from concourse._compat import cdiv

F32 = mybir.dt.float32
BF16 = mybir.dt.bfloat16
I16 = mybir.dt.int16
I32 = mybir.dt.int32

NEG_BIG = -1.0e38


# ---------------------------------------------------------------- host prep

def _wrap_idx(flat):
    """j-major flat int16 idx list [n] -> wrapped SBUF layout [128, n//16].

    dma_gather consumes idx i from wrapped[i % 16, i // 16]; the 16-row
    pattern is replicated to all 128 partitions.
    """
    n = len(flat)
    assert n % 128 == 0
    w = np.zeros((16, n // 16), np.int16)
    w[np.arange(n) % 16, np.arange(n) // 16] = flat
    return np.tile(w, (8, 1))


def prep(x, edge_index, batch, n_graphs, n_cores=8, j_cap=5, n_ag_chunks=4,
         mid_base=True):
    """All index-space preprocessing. Returns (meta, per-core arrays).

    - The x table fed to layer 1 is host-prescaled (xs = dinv * x) with a
      trailing zero row at index N, so pad slots gather exact zeros and the
      on-device accumulate is a plain sum for both layers.
    - The reference's added self-loops are NOT emitted as gather slots; the
      kernel adds the local (permuted-sequential) row per tile instead.
    - The h1 table is laid out AllGather-chunk-major: chunk k holds rows
      [NC * r0_k, NC * r1_k) as [core][local row] so each chunked AllGather
      writes a contiguous range.
    """
    N = x.shape[0]
    NR = N // n_cores                      # real nodes per core
    LV = int(cdiv(NR, 128)) * 128          # padded nodes per core
    T = LV // 128                          # tiles per core
    NP = LV * n_cores                      # padded total
    BASE1 = (N + 1) // 2 if mid_base else 0   # xs-table base row
    BASE2 = NP // 2 if mid_base else 0     # h1-table base row
    assert max(N + 1 - BASE1, BASE1, NP - BASE2, BASE2, NR + 1) <= 32767

    src_e = np.asarray(edge_index[0])
    dst_e = np.asarray(edge_index[1])
    deg = np.bincount(dst_e, minlength=N).astype(np.int64) + 1  # + self-loop
    dinv = (1.0 / np.sqrt(deg.astype(np.float32))).astype(np.float32)

    # group non-self edges by dst
    order = np.argsort(dst_e, kind="stable")
    src_s = src_e[order]
    starts = np.searchsorted(dst_e[order], np.arange(N))
    ends = np.searchsorted(dst_e[order], np.arange(N) + 1)

    # per-core degree-sorted permutation; perm[c][l] = orig id, -1 = pad
    perm = np.full((n_cores, LV), -1, np.int64)
    for c in range(n_cores):
        lo = NR * c
        perm[c, :NR] = np.argsort(-deg[lo:lo + NR], kind="stable") + lo

    # AllGather chunk boundaries (in tiles -> local rows); smaller last
    # chunk so the post-L1 exposure is short
    if n_ag_chunks == 4 and T == 49:
        bt = [0, 14, 27, 40, 49]
    else:
        bt = [round(k * T / n_ag_chunks) for k in range(n_ag_chunks + 1)]
    ag_rows = [(bt[k] * 128, bt[k + 1] * 128) for k in range(n_ag_chunks)]
    # orig id -> h1-table row (chunk-major AllGather layout)
    row_of = np.zeros(LV, np.int64)        # local row -> table row offset fn
    for (r0, r1) in ag_rows:
        row_of[r0:r1] = n_cores * r0 + np.arange(r1 - r0)
    chunk_len = np.zeros(LV, np.int64)
    for (r0, r1) in ag_rows:
        chunk_len[r0:r1] = r1 - r0
    perm_row = np.full(N, -1, np.int64)   # orig id -> h1-table row
    for c in range(n_cores):
        loc = np.arange(NR)
        perm_row[perm[c, :NR]] = row_of[loc] + c * chunk_len[loc]

    # J_t per tile (max over cores) of NON-SELF in-degree
    nsd = ends - starts
    Jt = np.zeros(T, np.int64)
    for c in range(n_cores):
        for t in range(T):
            ids = perm[c, t * 128:(t + 1) * 128]
            ids = ids[ids >= 0]
            if len(ids):
                Jt[t] = max(Jt[t], nsd[ids].max())
    Jt = np.maximum(Jt, 1)
    sumJ = int(Jt.sum())
    S = np.concatenate([[0], np.cumsum(Jt)]).astype(np.int64)  # tile slot starts
    K = int(cdiv(sumJ, j_cap))            # flat gathers of width j_cap

    # pad rows: any pad node's permuted row (h1' there is forced to 0);
    # the globally-last pad row lands at table row NP - 1 in every layout.
    pad_row2 = NP - 1 if NP > N else None
    assert pad_row2 is not None, "need at least one pad node for L2 padding"

    # per-core slot tables
    per_core = []
    for c in range(n_cores):
        idx1 = np.zeros((sumJ * 128,), np.int16)
        idx2 = np.zeros((sumJ * 128,), np.int16)
        dinv_dst = np.zeros((128, T), np.float32)
        off = 0
        for t in range(T):
            J = int(Jt[t])
            for p in range(128):
                n = perm[c, t * 128 + p]
                if n >= 0:
                    dinv_dst[p, t] = dinv[n]
                    ss = src_s[starts[n]:ends[n]]
                    nj = len(ss)
                    sl = (off + np.arange(nj)) * 128 + p
                    idx1[sl] = (ss - BASE1).astype(np.int16)
                    idx2[sl] = (perm_row[ss] - BASE2).astype(np.int16)
                else:
                    nj = 0
                # pad slots gather exact-zero rows
                if nj < J:
                    sl = (off + np.arange(nj, J)) * 128 + p
                    idx1[sl] = N - BASE1          # zero row of xs
                    idx2[sl] = pad_row2 - BASE2   # zero row
            off += J
        assert off == sumJ

        # wrapped layout per flat gather (slots packed across tile
        # boundaries).  Each gather gets one trailing sacrificial block
        # (the final descriptor flakily skips its data write): slot 0 of
        # the block is a valid row, the rest are negative, which the
        # gather ucode trims before descriptor-gen -- they cost neither
        # Q7 time nor DMA.
        pad_blk1 = np.full(128, -1, np.int16)
        pad_blk1[0] = N - BASE1
        pad_blk2 = np.full(128, -1, np.int16)
        pad_blk2[0] = pad_row2 - BASE2
        w1 = []
        w2 = []
        for k in range(K):
            w = min(j_cap, sumJ - k * j_cap)
            blk = slice(k * j_cap * 128, (k * j_cap + w) * 128)
            w1.append(_wrap_idx(np.concatenate([idx1[blk], pad_blk1])))
            w2.append(_wrap_idx(np.concatenate([idx2[blk], pad_blk2])))
        fence_blk = np.full(128, -1, np.int16)
        fence_blk[0] = 0                       # row 0 of the last ag chunk
        w2.append(_wrap_idx(fence_blk))
        idx1_w = np.concatenate(w1, axis=1)
        idx2_w = np.concatenate(w2, axis=1)
        per_core.append(dict(idx1=idx1_w, idx2=idx2_w, dinv_dst=dinv_dst))

    # pooling: h2 rows are scattered into a [j-slot, local graph] layout,
    # then max-reduced over j-slots.  GP/Jp are maxed over cores (SPMD).
    glo = np.zeros(n_cores, np.int64)
    Gc = np.zeros(n_cores, np.int64)
    for c in range(n_cores):
        b = batch[NR * c:NR * (c + 1)]
        glo[c] = b.min()
        Gc[c] = b.max() - b.min() + 1
    GP = int(Gc.max())
    assert GP <= 128
    # member slot j for each local permuted row, split into two tile
    # halves (A: tiles < Ts, B: rest) so the A half's max-reduction can
    # overlap the tail of layer 2
    Ts = (2 * T) // 3
    jslot = []
    JpA = JpB = 0
    for c in range(n_cores):
        cntA = np.zeros(GP, np.int64)
        cntB = np.zeros(GP, np.int64)
        js = np.full(LV, -1, np.int64)
        gl = np.full(LV, -1, np.int64)
        half = np.zeros(LV, np.int64)
        for l in range(LV):
            node = perm[c, l]
            if node >= 0:
                g = int(batch[node] - glo[c])
                cnt = cntA if (l // 128) < Ts else cntB
                half[l] = 0 if (l // 128) < Ts else 1
                js[l] = cnt[g]
                gl[l] = g
                cnt[g] += 1
        jslot.append((js, gl, half))
        JpA = max(JpA, int(cntA.max()))
        JpB = max(JpB, int(cntB.max()))
    for c in range(n_cores):
        js, gl, half = jslot[c]
        scat_rows = np.zeros((128, T), np.int32)
        for l in range(LV):
            t = l // 128
            if js[l] >= 0:
                scat_rows[l % 128, t] = js[l] * GP + gl[l]
            else:
                scat_rows[l % 128, t] = (JpA if t < Ts else JpB) * GP
        per_core[c]["scat_rows"] = scat_rows
        scat = np.full(128, n_graphs, np.int64)
        scat[:int(Gc[c])] = glo[c] + np.arange(int(Gc[c]))
        per_core[c]["scat_g"] = scat.astype(np.int32)[:, None]

    meta = dict(N=N, NP=NP, LV=LV, T=T, NC=n_cores, BASE1=BASE1, BASE2=BASE2,
                sumJ=sumJ, S=S, K=K, j_cap=j_cap, n_graphs=n_graphs,
                dinv=dinv, GP=GP, JpA=JpA, JpB=JpB, Ts=Ts, ag_rows=ag_rows,
                perm=perm)
    return meta, per_core


# ---------------------------------------------------------------- bass build

def build(meta, CIN, HID, HMLP, NCL, n_queues=4):
    """Build the SPMD Bass program. All per-core variation flows via inputs."""
    m = meta
    T, NC = m["T"], m["NC"]
    N, NP, LV = m["N"], m["NP"], m["LV"]
    S, K, JCAP = m["S"], m["K"], m["j_cap"]
    sumJ = m["sumJ"]
    GP, ag_rows = m["GP"], m["ag_rows"]
    JpA, JpB, Ts = m["JpA"], m["JpB"], m["Ts"]
    NG = m["n_graphs"]
    NGT = cdiv(NG, 128)          # pooled tiles (4)
    n_chunk_cols = sumJ + K

    nc = bacc.Bacc("TRN2", target_bir_lowering=False, debug=False,
                   num_devices=NC, num_swdge_queues=n_queues)
    qctr = [0]

    def next_q(avoid0=False):
        if avoid0:
            q = 1 + qctr[0] % (n_queues - 1)
        else:
            q = qctr[0] % n_queues
        qctr[0] += 1
        return q
    dt = mybir.dt

    # ---- inputs
    x_t = nc.dram_tensor("xs", [N + 1, CIN], BF16, kind="ExternalInput")
    xsp_t = nc.dram_tensor("xsp", [LV, CIN], BF16, kind="ExternalInput")
    idx1_t = nc.dram_tensor("idx1", [128, n_chunk_cols * 8], I16,
                            kind="ExternalInput")
    idx2_t = nc.dram_tensor("idx2", [128, (n_chunk_cols + 1) * 8], I16,
                            kind="ExternalInput")
    dinvd_t = nc.dram_tensor("dinv_dst", [128, T], F32, kind="ExternalInput")
    scatr_t = nc.dram_tensor("scat_rows", [128, T], I32, kind="ExternalInput")
    scat_t = nc.dram_tensor("scat_g", [128, 1], I32, kind="ExternalInput")
    W1_t = nc.dram_tensor("W1", [CIN, HID], F32, kind="ExternalInput")
    b1_t = nc.dram_tensor("b1", [1, HID], F32, kind="ExternalInput")
    W2_t = nc.dram_tensor("W2", [HID, HID], F32, kind="ExternalInput")
    b2_t = nc.dram_tensor("b2", [1, HID], F32, kind="ExternalInput")
    fcW1_t = nc.dram_tensor("fcW1", [HID, HMLP], F32, kind="ExternalInput")
    fcb1_t = nc.dram_tensor("fcb1", [1, HMLP], F32, kind="ExternalInput")
    fcW2_t = nc.dram_tensor("fcW2", [HMLP, NCL], F32, kind="ExternalInput")
    fcb2_t = nc.dram_tensor("fcb2", [1, NCL], F32, kind="ExternalInput")
    out_t = nc.dram_tensor("out", [NG, NCL], F32, kind="ExternalOutput")

    KB1 = CIN // 128    # K blocks layer1 (1)
    KB2 = HID // 128    # K blocks layer2 (2)
    KBM = HID // 128    # fc1 K blocks (2)

    with tile.TileContext(nc) as tc:
        with (
            tc.tile_pool(name="const", bufs=1) as cpool,
            tc.tile_pool(name="gath", bufs=8) as gpool,
            tc.tile_pool(name="poolld", bufs=2) as ppool,
            tc.tile_pool(name="work", bufs=4) as wpool,
            tc.tile_pool(name="outp", bufs=3) as opool,
            tc.tile_pool(name="tp_ps", bufs=2, space="PSUM") as tp_ps,
            tc.tile_pool(name="mm_ps", bufs=2, space="PSUM") as mm_ps,
            tc.tile_pool(name="dram", bufs=1, space="DRAM") as dr,
        ):
            # ---- constants / weights to SBUF
            ident = cpool.tile([128, 128], F32)
            make_identity(nc, ident[:])
            ones = cpool.tile([1, 128], F32)
            nc.vector.memset(ones[:], 1.0)
            negbig = cpool.tile([128, HID], BF16)
            nc.vector.memset(negbig[:], NEG_BIG)

            W1_sb = cpool.tile([128, KB1, HID], F32)
            for k in range(KB1):
                nc.sync.dma_start(out=W1_sb[:, k, :],
                                  in_=W1_t[k * 128:(k + 1) * 128, :])
            W2_sb = cpool.tile([128, KB2, HID], F32)
            for k in range(KB2):
                nc.sync.dma_start(out=W2_sb[:, k, :],
                                  in_=W2_t[k * 128:(k + 1) * 128, :])
            fcW1_sb = cpool.tile([128, KBM, HMLP], F32)
            for k in range(KBM):
                nc.sync.dma_start(out=fcW1_sb[:, k, :],
                                  in_=fcW1_t[k * 128:(k + 1) * 128, :])
            fcW2_sb = cpool.tile([128, NCL], F32)
            nc.sync.dma_start(out=fcW2_sb[:], in_=fcW2_t[:, :])
            b1_sb = cpool.tile([1, HID], F32)
            nc.sync.dma_start(out=b1_sb[:], in_=b1_t[:, :])
            b2_sb = cpool.tile([1, HID], F32)
            nc.sync.dma_start(out=b2_sb[:], in_=b2_t[:, :])
            fcb1_sb = cpool.tile([1, HMLP], F32)
            nc.sync.dma_start(out=fcb1_sb[:], in_=fcb1_t[:, :])
            fcb2_sb = cpool.tile([1, NCL], F32)
            nc.sync.dma_start(out=fcb2_sb[:], in_=fcb2_t[:, :])

            idx1_sb = cpool.tile([128, n_chunk_cols * 8], I16)
            _head = min(12 * 7 * 8, n_chunk_cols * 8)
            nc.sync.dma_start(out=idx1_sb[:, 0:_head],
                              in_=idx1_t[:, 0:_head])
            nc.sync.dma_start(out=idx1_sb[:, _head:],
                              in_=idx1_t[:, _head:])
            idx2_sb = cpool.tile([128, (n_chunk_cols + 1) * 8], I16)
            nc.sync.dma_start(out=idx2_sb[:], in_=idx2_t[:, :])
            dinvd_sb = cpool.tile([128, T], F32)
            nc.sync.dma_start(out=dinvd_sb[:], in_=dinvd_t[:, :])
            scatr_sb = cpool.tile([128, T], I32)
            nc.sync.dma_start(out=scatr_sb[:], in_=scatr_t[:, :])
            scat_sb = cpool.tile([128, 1], I32)
            nc.sync.dma_start(out=scat_sb[:], in_=scat_t[:, :])

            # ---- internal DRAM
            h1_shard = dr.tile([LV, HID], BF16)
            # AllGather chunk outputs allocated back-to-back (bump
            # allocator, sizes 4KB-multiples) so a spanning alias tensor
            # can serve as the layer-2 gather table -- no copy needed.
            ag_out = []
            for agk, (r0, r1) in enumerate(ag_rows):
                agt = nc.dram_tensor(f"ag_out{agk}", [NC * (r1 - r0), HID],
                                     BF16, addr_space="Shared")
                ag_out.append(agt)
            h1_table = nc.dram_tensor("h1_tbl_alias", [NP, HID], BF16,
                                      addr_space="Shared")
            _mls = [nc.lookup_mls(a) for a in ag_out]
            _mlocs = [s.memorylocations[0] for s in _mls]
            for _i in range(1, len(_mlocs)):
                _expect = _mlocs[_i - 1].addr + \
                    NC * (ag_rows[_i - 1][1] - ag_rows[_i - 1][0]) * HID * 2
                assert _mlocs[_i].addr == _expect, \
                    (_i, _mlocs[_i].addr, _expect)
            _aml = nc.lookup_mls(h1_table).memorylocations[0]
            _aml.addr = _mlocs[0].addr
            JPGA = (JpA + 1) * GP          # pool layout rows (+dump space)
            JPGB = (JpB + 1) * GP
            pool_a = dr.tile([JPGA, HID], BF16)
            pool_b = dr.tile([JPGB, HID], BF16)
            pool_scat = dr.tile([NG + 1, HID], BF16)
            pool_red = dr.tile([NG, HID], BF16, addr_space="Shared")

            # init pool_scat table to NEG_BIG
            for i in range(cdiv(NG + 1, 128)):
                r0 = i * 128
                r1 = min(r0 + 128, NG + 1)
                nc.sync.dma_start(out=pool_scat[r0:r1, :],
                                  in_=negbig[0:r1 - r0, :])

            # chunked AllGather: chunk k fires once its h1 tiles are written
            ag_done = [False] * len(ag_rows)

            def fire_ag(k):
                r0, r1 = ag_rows[k]
                nc.gpsimd.collective_compute(
                    "AllGather", mybir.AluOpType.bypass,
                    replica_groups=[list(range(NC))],
                    ins=[h1_shard[r0:r1, :]],
                    outs=[ag_out[k][:, :]])
                ag_done[k] = True

            # pooled accumulator + per-half pool reduction (max over
            # j-slots of a scatter layout)
            pooled = wpool.tile([128, HID], BF16, tag="pooled")
            nc.vector.memset(pooled[:], NEG_BIG)

            def pool_reduce_chunk(pl, j0, jc):
                pt = ppool.tile([128, 16, HID], BF16, tag="pool")
                dv = pl[j0 * GP:(j0 + jc) * GP, :].rearrange(
                    "(j g) c -> g j c", j=jc)
                nc.sync.dma_start(out=pt[0:GP, 0:jc, :], in_=dv)
                red = wpool.tile([128, HID], BF16, tag="red2")
                pv = pt[0:GP, 0:jc, :].rearrange("g j c -> g c j")
                nc.vector.tensor_reduce(out=red[0:GP, :], in_=pv,
                                        axis=mybir.AxisListType.X,
                                        op=mybir.AluOpType.max)
                nc.vector.tensor_max(out=pooled[0:GP, :],
                                     in0=pooled[0:GP, :],
                                     in1=red[0:GP, :])

            def pool_half_reduce(pl, Jp):
                JC = 16
                for j0 in range(0, Jp, JC):
                    pool_reduce_chunk(pl, j0, min(JC, Jp - j0))

            # ---------------- layer helper
            def gcn_layer(layer):
                if layer == 1:
                    C = CIN
                    idx_sb = idx1_sb
                    table_ap = x_t[:, :]
                    KB, W_sb, b_sb = KB1, W1_sb, b1_sb
                else:
                    C = HID
                    idx_sb = idx2_sb
                    table_ap = h1_table[:, :]
                    KB, W_sb, b_sb = KB2, W2_sb, b2_sb
                base = m["BASE1"] if layer == 1 else m["BASE2"]

                gtag = "g1" if layer == 1 else "g2"
                gbufs = {}
                emitted = [-1]

                def emit_gather(k):
                    w = min(JCAP, sumJ - k * JCAP)
                    g = gpool.tile([128, 8, C], BF16, tag=gtag)
                    icol = k * (JCAP + 1)
                    nc.gpsimd.dma_gather(
                        g[:, 0:w + 1, 0:C],
                        table_ap[base:, :],
                        idx_sb[:, icol * 8:(icol + w + 1) * 8],
                        (w + 1) * 128, (w + 1) * 128, C,
                        queue_num=next_q())
                    gbufs[k] = g
                    emitted[0] = k

                for t in range(T):
                    acc = wpool.tile([128, HID], F32, tag="acc")
                    s0, s1 = int(S[t]), int(S[t + 1])
                    k0, k1 = s0 // JCAP, (s1 - 1) // JCAP
                    first = True
                    for k in range(k0, k1 + 1):
                        if k > emitted[0]:
                            emit_gather(k)
                        g = gbufs[k]
                        a = max(s0, k * JCAP) - k * JCAP
                        b = min(s1, k * JCAP + JCAP) - k * JCAP
                        gv = g[:, a:b, 0:C].rearrange("p j c -> p c j")
                        if first:
                            nc.vector.tensor_reduce(
                                out=acc[:, 0:C], in_=gv,
                                axis=mybir.AxisListType.X,
                                op=mybir.AluOpType.add)
                        else:
                            red = wpool.tile([128, HID], F32, tag="red")
                            nc.vector.tensor_reduce(
                                out=red[:, 0:C], in_=gv,
                                axis=mybir.AxisListType.X,
                                op=mybir.AluOpType.add)
                            nc.vector.tensor_add(
                                out=acc[:, 0:C], in0=acc[:, 0:C],
                                in1=red[:, 0:C])
                        first = False
                    for kk in list(gbufs):
                        if kk < k0:
                            del gbufs[kk]

                    # self-loop term: local (permuted-sequential) rows
                    sl = wpool.tile([128, HID], BF16, tag="self")
                    if layer == 1:
                        nc.sync.dma_start(
                            out=sl[:, 0:C],
                            in_=xsp_t[t * 128:(t + 1) * 128, :])
                    else:
                        nc.sync.dma_start(
                            out=sl[:, 0:C],
                            in_=h1_shard[t * 128:(t + 1) * 128, :])
                    nc.vector.tensor_add(out=acc[:, 0:C], in0=acc[:, 0:C],
                                         in1=sl[:, 0:C])

                    # dst-side dinv scaling (Scalar engine; DVE is loaded)
                    nc.scalar.activation(
                        out=acc[:, 0:C], in_=acc[:, 0:C],
                        func=mybir.ActivationFunctionType.Copy,
                        scale=dinvd_sb[:, t:t + 1])

                    # transpose -> lhsT blocks
                    accT = wpool.tile([128, KB, 128], F32, tag="accT")
                    for k in range(KB):
                        tps = tp_ps.tile([128, 128], F32, tag="tp")
                        nc.tensor.transpose(out=tps[:],
                                            in_=acc[:, k * 128:(k + 1) * 128],
                                            identity=ident[:])
                        nc.scalar.activation(
                            out=accT[:, k, :], in_=tps[:],
                            func=mybir.ActivationFunctionType.Copy)

                    # matmul: bias + sum_k accT_k.T @ W_k
                    mm = mm_ps.tile([128, HID], F32, tag="mm")
                    nc.tensor.matmul(out=mm[:], lhsT=ones[0:1, :],
                                     rhs=b_sb[0:1, :], start=True, stop=False)
                    for k in range(KB):
                        nc.tensor.matmul(out=mm[:], lhsT=accT[:, k, :],
                                         rhs=W_sb[:, k, :],
                                         start=False, stop=(k == KB - 1))

                    if layer == 1:
                        # h1' = relu(dinv * (aggW + b)) = dinv * relu(aggW+b)
                        h = opool.tile([128, HID], BF16, tag="h")
                        nc.scalar.activation(
                            out=h[:], in_=mm[:],
                            func=mybir.ActivationFunctionType.Relu,
                            scale=dinvd_sb[:, t:t + 1])
                        nc.sync.dma_start(
                            out=h1_shard[t * 128:(t + 1) * 128, :], in_=h[:])
                        # fire any AllGather chunk whose rows are written
                        # (pipeline lag: wait 12 tiles past the boundary)
                        for k, (r0, r1) in enumerate(ag_rows):
                            if not ag_done[k] and (t + 1) * 128 >= r1 + 1024:
                                fire_ag(k)
                    else:
                        h = opool.tile([128, HID], BF16, tag="h")
                        nc.scalar.activation(
                            out=h[:], in_=mm[:],
                            func=mybir.ActivationFunctionType.Relu)
                        # scatter rows into the pooling [j-slot, graph] layout
                        nc.gpsimd.indirect_dma_start(
                            out=(pool_a if t < Ts else pool_b)[:, :],
                            out_offset=bass.IndirectOffsetOnAxis(
                                ap=scatr_sb[:, t:t + 1], axis=0),
                            in_=h[:], in_offset=None)
                        if layer == 2 and t >= Ts - 1:
                            ja = ((t - Ts + 1) // 3) * 16
                            if (t - Ts + 1) % 3 == 0 and ja < JpA:
                                pool_reduce_chunk(pool_a, ja,
                                                  min(16, JpA - ja))

            # ---------------- layer 1 + allgather
            gcn_layer(1)
            for k in range(len(ag_rows)):
                if not ag_done[k]:
                    fire_ag(k)
            # register each AllGather as a tracked writer of the aliased
            # gather table: 1-row self-copies (same physical bytes) whose
            # RAW dep is the collective -- every layer-2 gather then waits
            # on all four collectives via the table's writer set
            for k, (r0, r1) in enumerate(ag_rows):
                nc.sync.dma_start(
                    out=h1_table[NC * r0:NC * r0 + 1, :],
                    in_=ag_out[k][0:1, :])
            # pool layout init runs in the L1 drain window (must precede
            # the first L2 scatter)
            for pl, rows in ((pool_a, JPGA), (pool_b, JPGB)):
                for i in range(cdiv(rows, 128)):
                    r0 = i * 128
                    r1 = min(r0 + 128, rows)
                    nc.sync.dma_start(out=pl[r0:r1, :],
                                      in_=negbig[0:r1 - r0, :])

            # ---------------- layer 2
            gcn_layer(2)

            # ---------------- pooling tail: remaining A chunks, B half,
            # scatter + allreduce
            ja0 = (((T - 1) - Ts + 1) // 3 + 1) * 16
            for j0 in range(ja0, JpA, 16):
                pool_reduce_chunk(pool_a, j0, min(16, JpA - j0))
            pool_half_reduce(pool_b, JpB)
            nc.gpsimd.indirect_dma_start(
                out=pool_scat[:, :],
                out_offset=bass.IndirectOffsetOnAxis(ap=scat_sb[:, 0:1],
                                                     axis=0),
                in_=pooled[:],
                in_offset=None)
            nc.gpsimd.collective_compute(
                "AllReduce", mybir.AluOpType.max,
                replica_groups=[list(range(NC))],
                ins=[pool_scat[0:NG, :]], outs=[pool_red[:, :]])

            # ---------------- MLP + log_softmax (replicated)
            gT = wpool.tile([128, KBM, NGT * 128], F32, tag="gT")
            for i in range(NGT):
                gtile_bf = wpool.tile([128, HID], BF16, tag="gtileb")
                gtile = wpool.tile([128, HID], F32, tag="gtile")
                r0, r1 = i * 128, min((i + 1) * 128, NG)
                if r1 - r0 < 128:
                    nc.vector.memset(gtile[:], 0.0)
                nc.sync.dma_start(out=gtile_bf[0:r1 - r0, :],
                                  in_=pool_red[r0:r1, :])
                nc.vector.tensor_copy(out=gtile[0:r1 - r0, :],
                                      in_=gtile_bf[0:r1 - r0, :])
                for k in range(KBM):
                    tps = tp_ps.tile([128, 128], F32, tag="tp")
                    nc.tensor.transpose(out=tps[:],
                                        in_=gtile[:, k * 128:(k + 1) * 128],
                                        identity=ident[:])
                    nc.vector.tensor_copy(out=gT[:, k, i * 128:(i + 1) * 128],
                                          in_=tps[:])
            o1T = wpool.tile([128, NGT * 128], F32, tag="o1T")
            for i in range(NGT):
                mm1 = mm_ps.tile([128, HMLP], F32, tag="mm")
                nc.tensor.matmul(out=mm1[:], lhsT=ones[0:1, :],
                                 rhs=fcb1_sb[0:1, :], start=True, stop=False)
                for k in range(KBM):
                    nc.tensor.matmul(out=mm1[:],
                                     lhsT=gT[:, k, i * 128:(i + 1) * 128],
                                     rhs=fcW1_sb[:, k, :],
                                     start=False, stop=(k == KBM - 1))
                o1 = wpool.tile([128, HMLP], F32, tag="o1")
                nc.scalar.activation(out=o1[:], in_=mm1[:],
                                     func=mybir.ActivationFunctionType.Relu)
                tps = tp_ps.tile([128, 128], F32, tag="tp")
                nc.tensor.transpose(out=tps[0:HMLP, :], in_=o1[:],
                                    identity=ident[:])
                nc.vector.tensor_copy(out=o1T[0:HMLP, i * 128:(i + 1) * 128],
                                      in_=tps[0:HMLP, :])
            for i in range(NGT):
                mm2 = mm_ps.tile([128, NCL], F32, tag="mm2")
                nc.tensor.matmul(out=mm2[:], lhsT=ones[0:1, :],
                                 rhs=fcb2_sb[0:1, :], start=True, stop=False)
                nc.tensor.matmul(out=mm2[:],
                                 lhsT=o1T[0:HMLP, i * 128:(i + 1) * 128],
                                 rhs=fcW2_sb[0:HMLP, :],
                                 start=False, stop=True)
                # log_softmax rows
                mx = wpool.tile([128, 1], F32, tag="mx")
                nc.vector.tensor_reduce(out=mx[:], in_=mm2[:],
                                        axis=mybir.AxisListType.X,
                                        op=mybir.AluOpType.max)
                tsh = wpool.tile([128, NCL], F32, tag="tsh")
                nc.vector.tensor_scalar(
                    out=tsh[:], in0=mm2[:], scalar1=mx[:, 0:1], scalar2=None,
                    op0=mybir.AluOpType.subtract)
                ex = wpool.tile([128, NCL], F32, tag="ex")
                nc.scalar.activation(out=ex[:], in_=tsh[:],
                                     func=mybir.ActivationFunctionType.Exp)
                sm = wpool.tile([128, 1], F32, tag="sm")
                nc.vector.tensor_reduce(out=sm[:], in_=ex[:],
                                        axis=mybir.AxisListType.X,
                                        op=mybir.AluOpType.add)
                ls = wpool.tile([128, 1], F32, tag="ls")
                nc.scalar.activation(out=ls[:], in_=sm[:],
                                     func=mybir.ActivationFunctionType.Ln)
                oo = opool.tile([128, NCL], F32, tag="oo")
                nc.vector.tensor_scalar(
                    out=oo[:], in0=tsh[:], scalar1=ls[:, 0:1], scalar2=None,
                    op0=mybir.AluOpType.subtract)
                r0, r1 = i * 128, min((i + 1) * 128, NG)
                nc.sync.dma_start(out=out_t[r0:r1, :], in_=oo[0:r1 - r0, :])

    nc.compile()
    return nc


# ---------------------------------------------------------------- entry

def _ensure_ntff_hook():
    """Install the axon NTFF profile hook if the image's antenv lacks it.

    Dev-only (trace=True): lets run_bass_kernel_spmd return exec_time_ns.
    """
    import sys as _sys
    import types as _types
    try:
        from antenv.axon_hooks import get_axon_ntff_profile_hook  # noqa
        return
    except ImportError:
        pass
    try:
        _sys.path.insert(0, "/root/.axon_site")
        from trn_agent_boot.trn_boot import _ntff_profile_via_ctypes
        hook = _ntff_profile_via_ctypes("/opt/axon/libaxon_pjrt.so")
        mod = _types.ModuleType("antenv.axon_hooks")
        mod._hook = hook
        mod.get_axon_ntff_profile_hook = lambda: mod._hook
        mod.set_axon_ntff_profile_hook = lambda h: setattr(mod, "_hook", h)
        _sys.modules["antenv.axon_hooks"] = mod
        # artifact upload needs a bucket; degrade to no-op on failure
        _orig_upload = bass_utils.upload_artifacts

        def _safe_upload(tmpdir):
            try:
                return _orig_upload(tmpdir)
            except Exception:
                return tmpdir
        bass_utils.upload_artifacts = _safe_upload
    except Exception:
        pass


def kernel(x, edge_index, batch, W1, b1, W2, b2, fcW1, fcb1, fcW2, fcb2,
           trace=False):
    if trace:
        _ensure_ntff_hook()
    x = np.asarray(x, np.float32)
    edge_index = np.asarray(edge_index, np.int64)
    batch = np.asarray(batch, np.int64)
    NG = 512
    meta, per_core = prep(x, edge_index, batch, NG)

    nc = build(meta, CIN=x.shape[1], HID=W1.shape[1], HMLP=fcW1.shape[1],
               NCL=fcW2.shape[1])

    import ml_dtypes
    xs = np.concatenate([meta["dinv"][:, None] * x,
                         np.zeros((1, x.shape[1]), np.float32)], axis=0)
    xs_bf = xs.astype(ml_dtypes.bfloat16)
    shared = dict(
        xs=xs_bf,
        W1=np.asarray(W1, np.float32), b1=np.asarray(b1, np.float32)[None, :],
        W2=np.asarray(W2, np.float32), b2=np.asarray(b2, np.float32)[None, :],
        fcW1=np.asarray(fcW1, np.float32),
        fcb1=np.asarray(fcb1, np.float32)[None, :],
        fcW2=np.asarray(fcW2, np.float32),
        fcb2=np.asarray(fcb2, np.float32)[None, :],
    )
    perm = meta["perm"]
    in_maps = []
    for c in range(meta["NC"]):
        d = dict(shared)
        pc = per_core[c]
        # permuted local xs rows (pads -> zero row N)
        pidx = np.where(perm[c] >= 0, perm[c], x.shape[0])
        d["xsp"] = xs_bf[pidx]
        d["idx1"] = pc["idx1"]
        d["idx2"] = pc["idx2"]
        d["dinv_dst"] = pc["dinv_dst"]
        d["scat_rows"] = pc["scat_rows"]
        d["scat_g"] = np.tile(pc["scat_g"], (1, 1))
        in_maps.append(d)

    res = bass_utils.run_bass_kernel_spmd(
        nc, in_maps, core_ids=list(range(meta["NC"])), trace=trace)
    out = res.results[0]["out"]
    kernel.last_exec_time_ns = res.exec_time_ns
    return out


kernel.last_exec_time_ns = None

